# revision 24
# baseline (speedup 1.0000x reference)
"""DiffJPEG Trainium2 Bass kernel (self-contained).

Pure data-parallel over 8 NeuronCores (4 images each). Per image the pipeline
is four matmul stages in a ds/std/ds/std chain (ds = data-stationary: image
data rides the PE stationary operand, a constant block-diagonal DCT matrix
streams as rhs and the layout transposes; std = constant-stationary weights,
data streams as a wide N=512 rhs):

  S1 ds  [row,col] -> [col,(I,u)] : vertical DCT (+RGB->YCC fold, 2x1 avg)
  S2 std -> [(J,v),(I,u)]         : horizontal DCT (+1/fq fold; 1x2 avg)
  quant: q=c*rho; round via ACT magic-add; diff-round correction on DVE
  S3 ds  -> [(I,u),col]           : horizontal iDCT (+fq fold; 2x upsample)
  S4 std -> [(I,x),col]           : vertical iDCT + YCC->RGB folded into PSUM
                                    accumulation with pre-scaled chroma weights

Precision: encode side (x, w1, w2, h1, S1/S2 matmuls) is fp16 — bf16 there
flips quantizer rounding decisions and fails tolerance; decode side is bf16.
Pixels are host-centered by -128/255 so the color rows annihilate the DC
offset (chroma rows sum to 0, Y row to 1), shrinking encode magnitudes;
decode +128 rides the S3 Y eviction as an ACT bias on u==0 partitions.
Quant runs as a software pipeline: q evicts each S2 PSUM immediately into
wide [128,1024] tiles; the diff-round stages are batched across all 3 wide
tiles (DVE/ACT interleave with no per-tile cross-engine stalls), and images
are two-deep pipelined so the PE runs image m+1's S1/S2 during image m's
quant. GPSIMD/Pool is deliberately unused (real-HW cost far exceeds model).
"""
import sys
import numpy as np

sys.path.insert(0, "/opt/trn_rl_repo")

import ml_dtypes

BF16 = ml_dtypes.bfloat16
N_CORES = 8
IMGS = 4          # images per core
H = W = 512
MAGIC = 12582912.0  # 1.5*2**23: (x+M)-M == round-half-even(x) for |x|<2**22

# ---------------------------------------------------------------------------
# host-side constants
# ---------------------------------------------------------------------------
_xs = np.arange(8, dtype=np.float32)
_COS = np.cos((2 * _xs[:, None] + 1) * _xs[None, :] * np.pi / 16).astype(np.float32)
_alpha = np.array([1.0 / np.sqrt(2)] + [1.0] * 7, dtype=np.float32)
_Y_TABLE = np.array([
    [16, 11, 10, 16, 24, 40, 51, 61], [12, 12, 14, 19, 26, 58, 60, 55],
    [14, 13, 16, 24, 40, 57, 69, 56], [14, 17, 22, 29, 51, 87, 80, 62],
    [18, 22, 37, 56, 68, 109, 103, 77], [24, 35, 55, 64, 81, 104, 113, 92],
    [49, 64, 78, 87, 103, 121, 120, 101], [72, 92, 95, 98, 112, 100, 103, 99]],
    dtype=np.float32)
_C_TABLE = np.full((8, 8), 99.0, dtype=np.float32)
_C_TABLE[:4, :4] = np.array([[17, 18, 24, 47], [18, 21, 26, 66],
                             [24, 26, 56, 99], [47, 66, 99, 99]], dtype=np.float32)
_RGB2YCC = np.array([[0.299, 0.587, 0.114],
                    [-0.168736, -0.331264, 0.5],
                    [0.5, -0.418688, -0.081312]], dtype=np.float32)
_YCC2RGB = np.array([[1.0, 0.0, 1.402],
                    [1.0, -0.344136, -0.714136],
                    [1.0, 1.772, 0.0]], dtype=np.float32)


def _bd(M, n):
    r, c = M.shape
    out = np.zeros((r * n, c * n), dtype=np.float64)
    for i in range(n):
        out[i * r:(i + 1) * r, i * c:(i + 1) * c] = M
    return out


def _base_mats():
    Av = (_COS.astype(np.float64) * 0.5 * _alpha.astype(np.float64)[None, :])  # [x,u]
    Avi = Av.T.copy()                                   # [u,x]
    Avs = np.zeros((16, 8))                             # subsample fwd
    for x2 in range(16):
        Avs[x2] = Av[x2 // 2] / 2.0
    Avu = np.zeros((8, 16))                             # upsample inv
    for x2 in range(16):
        Avu[:, x2] = Avi[:, x2 // 2]
    return Av, Avi, Avs, Avu


def build_core_inputs(x_core, quality_core):
    """x_core [IMGS,3,512,512] f32, quality_core [IMGS] f32 -> in_map dict."""
    Av, Avi, Avs, Avu = _base_mats()
    f32 = np.float32
    bd16v = _bd(Av, 16)        # [128,128] 1D fwd (vertical or horizontal)
    bd8s = _bd(Avs, 8)         # [128,64]  fwd subsampled
    bd16i = _bd(Avi, 16)       # [128,128] 1D inverse
    bd8u = _bd(Avu, 8)         # [64,128]  inverse upsampling
    bd8u2 = np.concatenate([bd8u, bd8u], axis=0)        # [128,128] parity-stacked

    # S1 rhs per plane: out cols = [Y-Iu 128 | cb-I'u 64 | cr-I'u 64]
    w1 = np.stack([
        np.concatenate([255.0 * _RGB2YCC[0, p] * bd16v,
                        255.0 * _RGB2YCC[1, p] * bd8s,
                        255.0 * _RGB2YCC[2, p] * bd8s], axis=1).astype(np.float16)
        for p in range(3)])                                            # [3,128,256]

    fqs = []
    for q in np.asarray(quality_core, dtype=np.float64):
        factor = (5000.0 / q if q < 50.0 else 200.0 - 2.0 * q) / 100.0
        fqs.append(factor)

    w2y = np.stack([(bd16v / fq).astype(np.float16) for fq in fqs])   # [4,128,128]
    w2c = np.stack([(bd8s / fq).astype(np.float16) for fq in fqs])     # [4,128,64]
    w3y = np.stack([(bd16i * fq).astype(BF16) for fq in fqs])          # [4,128,128]
    w3c = np.stack([(bd8u2 * fq).astype(BF16) for fq in fqs])          # [4,128,128]
    w4y = (bd16i / 255.0).astype(BF16)                                 # [128,128]
    C = _YCC2RGB.astype(np.float64)
    w4cs = np.stack([(c * bd8u2 / 255.0).astype(BF16)
                     for c in (C[0, 2], C[1, 1], C[1, 2], C[2, 1])])   # [4,128,128]

    # quant patterns in [(J,v) partition, (I,u) free] layout:
    # value[p, f] = T[u(f%8), v(p%8)] -> tile T.T along partitions
    rho_y = np.tile((1.0 / _Y_TABLE).T, (16, 1)).astype(f32)           # [128,8]
    t_y = np.tile(_Y_TABLE.T, (16, 1)).astype(np.float16)
    rho_c = np.tile((1.0 / _C_TABLE).T, (16, 1)).astype(f32)
    t_c = np.tile(_C_TABLE.T, (16, 1)).astype(np.float16)

    mask = (np.arange(128) % 8 == 0).astype(f32)[:, None]
    # decode-side +128 on Y: bias on zy u==0 partitions through w4y's
    # Avi[0,x]/255 gain -> +0.5 on every output pixel
    zyb = (mask * (0.5 * 255.0 / float(Avi[0, 0]))).astype(f32)        # [128,1]

    # centered pixels: the color rows annihilate the 128 offset exactly
    # (chroma rows sum to 0, Y row to 1), shrinking all encode magnitudes
    xc = np.ascontiguousarray(x_core, dtype=np.float32) - np.float32(128.0 / 255.0)
    return {
        "x": xc.astype(np.float16),
        "w1": w1, "w2y": w2y, "w2c": w2c, "w3y": w3y, "w3c": w3c,
        "w4y": w4y, "w4cs": w4cs,
        "rho_y": rho_y, "t_y": t_y, "rho_c": rho_c, "t_c": t_c,
        "zyb": zyb,
    }


# ---------------------------------------------------------------------------
# bass program
# ---------------------------------------------------------------------------
def build_program(repeat=1):
    import concourse.bacc as bacc
    import concourse.mybir as mybir
    from concourse.tile import TileContext

    f32 = mybir.dt.float32
    f16 = mybir.dt.float16
    b16 = mybir.dt.bfloat16
    op = mybir.AluOpType
    AF = mybir.ActivationFunctionType

    nc = bacc.Bacc("TRN2", target_bir_lowering=False, debug=False,
                   enable_asserts=False, num_devices=N_CORES)

    x_d = nc.dram_tensor("x", [IMGS, 3, H, W], f16, kind="ExternalInput").ap()
    out_d = nc.dram_tensor("out", [IMGS, 3, H, W], b16, kind="ExternalOutput").ap()
    w1_d = nc.dram_tensor("w1", [3, 128, 256], f16, kind="ExternalInput").ap()
    w2y_d = nc.dram_tensor("w2y", [IMGS, 128, 128], f16, kind="ExternalInput").ap()
    w2c_d = nc.dram_tensor("w2c", [IMGS, 128, 64], f16, kind="ExternalInput").ap()
    w3y_d = nc.dram_tensor("w3y", [IMGS, 128, 128], b16, kind="ExternalInput").ap()
    w3c_d = nc.dram_tensor("w3c", [IMGS, 128, 128], b16, kind="ExternalInput").ap()
    w4y_d = nc.dram_tensor("w4y", [128, 128], b16, kind="ExternalInput").ap()
    w4cs_d = nc.dram_tensor("w4cs", [4, 128, 128], b16, kind="ExternalInput").ap()
    rho_y_d = nc.dram_tensor("rho_y", [128, 8], f32, kind="ExternalInput").ap()
    t_y_d = nc.dram_tensor("t_y", [128, 8], f16, kind="ExternalInput").ap()
    rho_c_d = nc.dram_tensor("rho_c", [128, 8], f32, kind="ExternalInput").ap()
    t_c_d = nc.dram_tensor("t_c", [128, 8], f16, kind="ExternalInput").ap()
    zyb_d = nc.dram_tensor("zyb", [128, 1], f32, kind="ExternalInput").ap()

    with TileContext(nc, trace_sim=False) as tc:
        with tc.tile_pool(name="consts", bufs=1) as cp, \
             tc.tile_pool(name="pix", bufs=6) as pixp, \
             tc.tile_pool(name="h1", bufs=8) as h1p, \
             tc.tile_pool(name="qq", bufs=9) as qp, \
             tc.tile_pool(name="tmp", bufs=4) as tp, \
             tc.tile_pool(name="r2", bufs=6) as r2p, \
             tc.tile_pool(name="zz", bufs=5) as zp, \
             tc.tile_pool(name="outp", bufs=6) as op_, \
             tc.tile_pool(name="ps", bufs=1, space="PSUM") as pp:

            def cload(ap_dram, shape, tag, dt_=b16):
                t = cp.tile(shape, dt_, tag=tag, name=tag)
                nc.sync.dma_start(out=t[:], in_=ap_dram)
                return t

            w1_s = [cload(w1_d[p], [128, 256], f"w1{p}", f16) for p in range(3)]
            w2y_s = [cload(w2y_d[m], [128, 128], f"w2y{m}", f16)
                     for m in range(IMGS)]
            w2c_s = [cload(w2c_d[m], [128, 64], f"w2c{m}", f16)
                     for m in range(IMGS)]
            w3y_s = [cload(w3y_d[m], [128, 128], f"w3y{m}") for m in range(IMGS)]
            w3c_s = [cload(w3c_d[m], [128, 128], f"w3c{m}") for m in range(IMGS)]
            w4y_s = cload(w4y_d, [128, 128], "w4y")
            w4cs_s = [cload(w4cs_d[k], [128, 128], f"w4cs{k}") for k in range(4)]
            rho_y_s = cload(rho_y_d, [128, 8], "rho_y", f32)
            t_y_s = cload(t_y_d, [128, 8], "t_y", f16)
            rho_c_s = cload(rho_c_d, [128, 8], "rho_c", f32)
            t_c_s = cload(t_c_d, [128, 8], "t_c", f16)
            zyb_s = cload(zyb_d, [128, 1], "zyb", f32)

            def bcast8(t):  # [128,8] const -> [128,64,8] step-0 broadcast (==512)
                return t[:, None, :].broadcast_to((128, 64, 8))

            def bcast8w(t):  # wide variant (==1024)
                return t[:, None, :].broadcast_to((128, 128, 8))

            def mm(out, lhsT, rhs, **kw):
                nc.tensor.matmul(out, lhsT=lhsT, rhs=rhs, **kw)

            def quant_front(ps_tile, rho_s, qw, half):
                """Evict psum coeffs as q = c*(1/T) into half of a wide tile."""
                nc.vector.tensor_tensor(out=qw[:, 512 * half:512 * (half + 1)],
                                        in0=ps_tile[:], in1=bcast8(rho_s),
                                        op=op.mult)

            def quant_back(qs):
                """Batched diff-round on wide [128,1024] tiles: stages run
                across all tiles so each engine always has independent work.
                qs: list of (q_wide, t_s); returns wide bf16 r2 tiles."""
                n = len(qs)
                t1s = [tp.tile([128, 1024], f32, tag="t1", name="t1") for _ in range(n)]
                dps = [tp.tile([128, 1024], f16, tag="dp", name="dp") for _ in range(n)]
                d2s = [tp.tile([128, 1024], f16, tag="d2", name="d2") for _ in range(n)]
                gs = [tp.tile([128, 1024], f16, tag="g", name="g") for _ in range(n)]
                rs = [tp.tile([128, 1024], f16, tag="r", name="r") for _ in range(n)]
                r2s = [r2p.tile([128, 1024], b16, tag="r2", name="r2")
                       for _ in range(n)]
                # t1 = q + MAGIC (fp32 store rounds -> MAGIC + round(q))
                for k, (q, _) in enumerate(qs):
                    nc.scalar.activation(out=t1s[k][:], in_=q[:], func=AF.Copy,
                                         bias=MAGIC)
                # dp = (t1 - MAGIC) - q = round(q) - q = -d
                for k, (q, _) in enumerate(qs):
                    nc.vector.scalar_tensor_tensor(
                        out=dps[k][:], in0=t1s[k][:], scalar=-MAGIC, in1=q[:],
                        op0=op.add, op1=op.subtract)
                for k in range(n):
                    nc.scalar.square(out=d2s[k][:], in_=dps[k][:])
                # g = (d2-1)*dp = d - d^3
                for k in range(n):
                    nc.vector.scalar_tensor_tensor(
                        out=gs[k][:], in0=d2s[k][:], scalar=1.0, in1=dps[k][:],
                        op0=op.subtract, op1=op.mult)
                # r = q - g = round(q) + d^3
                for k, (q, _) in enumerate(qs):
                    nc.vector.tensor_tensor(out=rs[k][:], in0=q[:], in1=gs[k][:],
                                            op=op.subtract)
                # r2 = r * T
                for k, (_, t_s) in enumerate(qs):
                    nc.vector.tensor_tensor(out=r2s[k][:], in0=rs[k][:],
                                            in1=bcast8w(t_s), op=op.mult)
                return r2s

            def _front(m):
                """pix DMA + S1 + S2 + q-eviction for image m."""
                # ---- load pixel planes (one DMA per plane) ----
                pixpl = [pixp.tile([128, 2048], f16, tag="pix", name="pix")
                         for _ in range(3)]
                for p in range(3):
                    nc.sync.dma_start(
                        out=pixpl[p][:].rearrange("p (i c) -> p i c", i=4, c=512),
                        in_=x_d[m, p].rearrange("(i p) c -> p i c", i=4, p=128))
                pix = [[pixpl[p][:, 512 * i:512 * (i + 1)] for i in range(4)]
                       for p in range(3)]

                # ---- S1 (ds): vertical DCT (+color fold); 2 psum banks per
                # c-chunk, bank b: [i=2b: Y128 cb64 cr64 | i=2b+1: ...]
                h1y = []   # sbuf [c-chunk, Iu 512]
                h1c = []   # sbuf [c-chunk, cb-I'u 256 | cr-I'u 256]
                for j in range(4):
                    banks = [pp.tile([128, 512], f32, tag="psA", name="psS1", bufs=2)
                             for _ in range(2)]
                    for i in range(4):
                        bank = banks[i // 2]
                        o0 = 256 * (i % 2)
                        for p in range(3):
                            mm(bank[:, o0:o0 + 256],
                               lhsT=pix[p][i][:, 128 * j:128 * (j + 1)],
                               rhs=w1_s[p][:],
                               start=(p == 0), stop=(p == 2))
                    ty = h1p.tile([128, 512], f16, tag="h1y", name="h1y")
                    tch = h1p.tile([128, 512], f16, tag="h1c", name="h1c")
                    for b in range(2):
                        v = banks[b][:].rearrange("p (i s) -> p i s", i=2, s=256)
                        nc.scalar.copy(
                            out=ty[:].rearrange("p (i s) -> p i s", i=4, s=128)
                                [:, 2 * b:2 * b + 2, :],
                            in_=v[:, :, 0:128])
                        nc.scalar.copy(
                            out=tch[:].rearrange("p (c i v) -> p c i v",
                                                 c=2, i=4, v=64)
                                [:, :, 2 * b:2 * b + 2, :],
                            in_=v[:, :, 128:256].rearrange("p i (c v) -> p c i v",
                                                           c=2, v=64))
                    h1y.append(ty)
                    h1c.append(tch)

                # ---- S2 (std): horizontal DCT -> coeffs [(J,v), (I,u)] ----
                qw = [qp.tile([128, 1024], f32, tag="q", name="q")
                      for _ in range(3)]
                for j in range(4):
                    psQ = pp.tile([128, 512], f32, tag="psB", name="psQ", bufs=2)
                    mm(psQ[:], lhsT=w2y_s[m][:], rhs=h1y[j][:],
                                     start=True, stop=True)
                    quant_front(psQ, rho_y_s, qw[j // 2], j % 2)
                # chroma: one [128,512] psum per j-pair b; rows 0:64 = cb,
                # 64:128 = cr (partition-offset matmul writes)
                for b in range(2):
                    psQ = pp.tile([128, 512], f32, tag="psB", name="psQc", bufs=2)
                    for ch in range(2):
                        for jj in range(2):
                            j = 2 * b + jj
                            mm(psQ[64 * ch:64 * ch + 64,
                                   256 * jj:256 * (jj + 1)],
                               lhsT=w2c_s[m][:],
                               rhs=h1c[j][:, 256 * ch:256 * (ch + 1)],
                               start=True, stop=True)
                    quant_front(psQ, rho_c_s, qw[2], b)
                return [(qw[0], t_y_s), (qw[1], t_y_s), (qw[2], t_c_s)]

            def _qb(m, qt):
                """diff-round for image m -> (r2y, r2cc) wide-tile views."""
                r2w = quant_back(qt)
                r2y = [r2w[j // 2][:, 512 * (j % 2):512 * (j % 2 + 1)]
                       for j in range(4)]
                r2cc = [r2w[2][:, 512 * b:512 * (b + 1)] for b in range(2)]
                return r2y, r2cc

            def _s34(m, r2y, r2cc):
                """S3 + S4 + store for image m."""

                # ---- S3 (ds): horizontal iDCT (+h-upsample) -> [(I,u), c] ----
                zy = []
                for i in range(4):
                    psZ = pp.tile([128, 512], f32, tag="psC", name="psZ", bufs=2)
                    for j in range(4):
                        mm(psZ[:, 128 * j:128 * (j + 1)],
                                         lhsT=r2y[j][:, 128 * i:128 * (i + 1)],
                                         rhs=w3y_s[m][:], start=True, stop=True)
                    t_ = zp.tile([128, 512], b16, tag="zy", name="zy")
                    # eviction carries the decode-side +128-on-Y as a
                    # per-partition bias on u==0 rows
                    nc.scalar.activation(out=t_[:], in_=psZ[:], func=AF.Identity,
                                         bias=zyb_s[:])
                    zy.append(t_)
                # chroma Z [I'u, c]: per channel 2 tiles (I'u-chunks)
                zc = [[], []]
                for ch in range(2):
                    for k in range(2):
                        psZ = pp.tile([128, 512], f32, tag="psC", name="psZc",
                                      bufs=2)
                        po = 64 * ch
                        for j in range(4):
                            fo = 256 * (j % 2) + 128 * k
                            mm(
                                psZ[:, 128 * j:128 * (j + 1)],
                                lhsT=r2cc[j // 2][po:po + 64, fo:fo + 128],
                                rhs=w3c_s[m][po:po + 64, :],
                                start=True, stop=True)
                        t_ = zp.tile([128, 512], b16, tag="zc", name="zc")
                        nc.scalar.copy(out=t_[:], in_=psZ[:])
                        zc[ch].append(t_)

                # ---- S4 (std): vertical iDCT with YCC->RGB folded into the
                # PSUM accumulation (chroma weights pre-scaled by the mix
                # coefficients), then clamp + store ----
                outpl = [op_.tile([128, 2048], b16, tag="o", name="o")
                         for _ in range(3)]
                for i in range(4):
                    po = 64 * (i % 2)
                    zcb = zc[0][i // 2][po:po + 64, :]
                    zcr = zc[1][i // 2][po:po + 64, :]
                    def clamp(pl, ps):
                        if pl >= 1:
                            # ACT evicts to bf16 (frees the PSUM bank fast);
                            # the clamp then runs in DVE 2x packed mode
                            ev = op_.tile([128, 512], b16, tag="ev", name="ev")
                            nc.scalar.copy(out=ev[:], in_=ps[:])
                            nc.vector.tensor_scalar(
                                out=outpl[pl][:, 512 * i:512 * (i + 1)],
                                in0=ev[:], scalar1=0.0, scalar2=1.0,
                                op0=op.max, op1=op.min)
                        else:
                            nc.vector.tensor_scalar(
                                out=outpl[pl][:, 512 * i:512 * (i + 1)],
                                in0=ps[:], scalar1=0.0, scalar2=1.0,
                                op0=op.max, op1=op.min)
                    psR = pp.tile([128, 512], f32, tag="psD", name="psR", bufs=2)
                    psG = pp.tile([128, 512], f32, tag="psD", name="psG", bufs=2)
                    mm(psR[:], lhsT=w4y_s[:], rhs=zy[i][:], start=True, stop=False)
                    mm(psR[:], lhsT=w4cs_s[0][po:po + 64, :], rhs=zcr,
                       start=False, stop=True)
                    mm(psG[:], lhsT=w4y_s[:], rhs=zy[i][:], start=True, stop=False)
                    mm(psG[:], lhsT=w4cs_s[1][po:po + 64, :], rhs=zcb,
                       start=False, stop=False)
                    clamp(0, psR)
                    mm(psG[:], lhsT=w4cs_s[2][po:po + 64, :], rhs=zcr,
                       start=False, stop=True)
                    psB = pp.tile([128, 512], f32, tag="psD", name="psB", bufs=2)
                    mm(psB[:], lhsT=w4y_s[:], rhs=zy[i][:], start=True, stop=False)
                    mm(psB[:], lhsT=w4cs_s[3][po:po + 64, :], rhs=zcb,
                       start=False, stop=True)
                    clamp(1, psG)
                    clamp(2, psB)
                for pl in range(3):
                    nc.sync.dma_start(
                        out=out_d[m, pl].rearrange("(i p) c -> p i c", i=4, p=128),
                        in_=outpl[pl][:].rearrange("p (i c) -> p i c", i=4, c=512))

            def _build_images():
                # three-deep software pipeline: diff-round of image m is
                # emitted ahead of image m+2's front so its DVE/ACT work
                # overlaps two images' worth of PE time before S3(m) needs it
                qts = [None] * IMGS
                r2s = [None] * IMGS
                qts[0] = _front(0)
                qts[1] = _front(1)
                r2s[0] = _qb(0, qts[0])
                for m in range(IMGS):
                    if m + 2 < IMGS:
                        qts[m + 2] = _front(m + 2)
                    _s34(m, *r2s[m])
                    r2s[m] = qts[m] = None
                    if m + 1 < IMGS:
                        r2s[m + 1] = _qb(m + 1, qts[m + 1])

            if repeat == 1:
                _build_images()
            else:
                with tc.For_i(0, repeat, 1):
                    _build_images()
    nc.compile()
    return nc


_NC_CACHE = {}


def _get_nc():
    if "nc" not in _NC_CACHE:
        _NC_CACHE["nc"] = build_program()
    return _NC_CACHE["nc"]


def kernel(x, quality):
    """Full inputs -> full output. Shards batch over 8 cores internally."""
    from concourse import bass_utils
    x = np.asarray(x, dtype=np.float32)
    quality = np.asarray(quality, dtype=np.float32)
    B = x.shape[0]
    assert B == N_CORES * IMGS, (B, N_CORES, IMGS)
    nc = _get_nc()
    in_maps = []
    for c in range(N_CORES):
        sl = slice(c * IMGS, (c + 1) * IMGS)
        in_maps.append(build_core_inputs(x[sl], quality[sl]))
    res = bass_utils.run_bass_kernel_spmd(nc, in_maps, core_ids=list(range(N_CORES)))
    outs = [np.asarray(res.results[c]["out"]).astype(np.float32)
            for c in range(N_CORES)]
    return np.concatenate(outs, axis=0)


# revision 25
# speedup vs baseline: 1.3014x; 1.3014x over previous
"""DiffJPEG Trainium2 Bass kernel (self-contained).

Pure data-parallel over 8 NeuronCores (4 images each). Per image the pipeline
is four matmul stages in a ds/std/ds/std chain (ds = data-stationary: image
data rides the PE stationary operand, a constant block-diagonal DCT matrix
streams as rhs and the layout transposes; std = constant-stationary weights,
data streams as a wide N=512 rhs):

  S1 ds  [row,col] -> [col,(I,u)] : vertical DCT (+RGB->YCC fold, 2x1 avg)
  S2 std -> [(J,v),(I,u)]         : horizontal DCT (+1/fq fold; 1x2 avg)
  quant: q=c*rho; round via ACT magic-add; diff-round correction on DVE
  S3 ds  -> [(I,u),col]           : horizontal iDCT (+fq fold; 2x upsample)
  S4 std -> [(I,x),col]           : vertical iDCT + YCC->RGB folded into PSUM
                                    accumulation with pre-scaled chroma weights

Precision: encode side (x, w1, w2, h1, S1/S2 matmuls) is fp16 — bf16 there
flips quantizer rounding decisions and fails tolerance; decode side is bf16.
Pixels are host-centered by -128/255 so the color rows annihilate the DC
offset (chroma rows sum to 0, Y row to 1), shrinking encode magnitudes;
decode +128 rides the S3 Y eviction as an ACT bias on u==0 partitions.
Quant runs as a software pipeline: q evicts each S2 PSUM immediately into
wide [128,1024] tiles; the diff-round stages are batched across all 3 wide
tiles (DVE/ACT interleave with no per-tile cross-engine stalls), and images
are two-deep pipelined so the PE runs image m+1's S1/S2 during image m's
quant. GPSIMD/Pool is deliberately unused (real-HW cost far exceeds model).
"""
import sys
import numpy as np

sys.path.insert(0, "/opt/trn_rl_repo")

import ml_dtypes

BF16 = ml_dtypes.bfloat16
N_CORES = 8
IMGS = 4          # images per core
H = W = 512
MAGIC = 12582912.0  # 1.5*2**23: (x+M)-M == round-half-even(x) for |x|<2**22

# ---------------------------------------------------------------------------
# host-side constants
# ---------------------------------------------------------------------------
_xs = np.arange(8, dtype=np.float32)
_COS = np.cos((2 * _xs[:, None] + 1) * _xs[None, :] * np.pi / 16).astype(np.float32)
_alpha = np.array([1.0 / np.sqrt(2)] + [1.0] * 7, dtype=np.float32)
_Y_TABLE = np.array([
    [16, 11, 10, 16, 24, 40, 51, 61], [12, 12, 14, 19, 26, 58, 60, 55],
    [14, 13, 16, 24, 40, 57, 69, 56], [14, 17, 22, 29, 51, 87, 80, 62],
    [18, 22, 37, 56, 68, 109, 103, 77], [24, 35, 55, 64, 81, 104, 113, 92],
    [49, 64, 78, 87, 103, 121, 120, 101], [72, 92, 95, 98, 112, 100, 103, 99]],
    dtype=np.float32)
_C_TABLE = np.full((8, 8), 99.0, dtype=np.float32)
_C_TABLE[:4, :4] = np.array([[17, 18, 24, 47], [18, 21, 26, 66],
                             [24, 26, 56, 99], [47, 66, 99, 99]], dtype=np.float32)
_RGB2YCC = np.array([[0.299, 0.587, 0.114],
                    [-0.168736, -0.331264, 0.5],
                    [0.5, -0.418688, -0.081312]], dtype=np.float32)
_YCC2RGB = np.array([[1.0, 0.0, 1.402],
                    [1.0, -0.344136, -0.714136],
                    [1.0, 1.772, 0.0]], dtype=np.float32)


def _bd(M, n):
    r, c = M.shape
    out = np.zeros((r * n, c * n), dtype=np.float64)
    for i in range(n):
        out[i * r:(i + 1) * r, i * c:(i + 1) * c] = M
    return out


def _base_mats():
    Av = (_COS.astype(np.float64) * 0.5 * _alpha.astype(np.float64)[None, :])  # [x,u]
    Avi = Av.T.copy()                                   # [u,x]
    Avs = np.zeros((16, 8))                             # subsample fwd
    for x2 in range(16):
        Avs[x2] = Av[x2 // 2] / 2.0
    Avu = np.zeros((8, 16))                             # upsample inv
    for x2 in range(16):
        Avu[:, x2] = Avi[:, x2 // 2]
    return Av, Avi, Avs, Avu


def build_core_inputs(x_core, quality_core):
    """x_core [IMGS,3,512,512] f32, quality_core [IMGS] f32 -> in_map dict."""
    Av, Avi, Avs, Avu = _base_mats()
    f32 = np.float32
    bd16v = _bd(Av, 16)        # [128,128] 1D fwd (vertical or horizontal)
    bd8s = _bd(Avs, 8)         # [128,64]  fwd subsampled
    bd16i = _bd(Avi, 16)       # [128,128] 1D inverse
    bd8u = _bd(Avu, 8)         # [64,128]  inverse upsampling
    bd8u2 = np.concatenate([bd8u, bd8u], axis=0)        # [128,128] parity-stacked

    # S1 rhs per plane: out cols = [Y-Iu 128 | cb-I'u 64 | cr-I'u 64]
    w1 = np.stack([
        np.concatenate([255.0 * _RGB2YCC[0, p] * bd16v,
                        255.0 * _RGB2YCC[1, p] * bd8s,
                        255.0 * _RGB2YCC[2, p] * bd8s], axis=1).astype(np.float16)
        for p in range(3)])                                            # [3,128,256]

    fqs = []
    for q in np.asarray(quality_core, dtype=np.float64):
        factor = (5000.0 / q if q < 50.0 else 200.0 - 2.0 * q) / 100.0
        fqs.append(factor)

    w2y = np.stack([(bd16v / fq).astype(np.float16) for fq in fqs])   # [4,128,128]
    w2c = np.stack([(bd8s / fq).astype(np.float16) for fq in fqs])     # [4,128,64]
    w3y = np.stack([(bd16i * fq).astype(BF16) for fq in fqs])          # [4,128,128]
    w3c = np.stack([(bd8u2 * fq).astype(BF16) for fq in fqs])          # [4,128,128]
    w4y = (bd16i / 255.0).astype(BF16)                                 # [128,128]
    C = _YCC2RGB.astype(np.float64)
    w4cs = np.stack([(c * bd8u2 / 255.0).astype(BF16)
                     for c in (C[0, 2], C[1, 1], C[1, 2], C[2, 1])])   # [4,128,128]

    # quant patterns in [(J,v) partition, (I,u) free] layout:
    # value[p, f] = T[u(f%8), v(p%8)] -> tile T.T along partitions
    rho_y = np.tile((1.0 / _Y_TABLE).T, (16, 1)).astype(f32)           # [128,8]
    t_y = np.tile(_Y_TABLE.T, (16, 1)).astype(f32)
    rho_c = np.tile((1.0 / _C_TABLE).T, (16, 1)).astype(f32)
    t_c = np.tile(_C_TABLE.T, (16, 1)).astype(f32)

    mask = (np.arange(128) % 8 == 0).astype(f32)[:, None]
    # decode-side +128 on Y: bias on zy u==0 partitions through w4y's
    # Avi[0,x]/255 gain -> +0.5 on every output pixel
    zyb = (mask * (0.5 * 255.0 / float(Avi[0, 0]))).astype(f32)        # [128,1]

    # centered pixels: the color rows annihilate the 128 offset exactly
    # (chroma rows sum to 0, Y row to 1), shrinking all encode magnitudes
    xc = np.ascontiguousarray(x_core, dtype=np.float32) - np.float32(128.0 / 255.0)
    return {
        "x": xc.astype(np.float16),
        "w1": w1, "w2y": w2y, "w2c": w2c, "w3y": w3y, "w3c": w3c,
        "w4y": w4y, "w4cs": w4cs,
        "rho_y": rho_y, "t_y": t_y, "rho_c": rho_c, "t_c": t_c,
        "zyb": zyb,
    }


# ---------------------------------------------------------------------------
# bass program
# ---------------------------------------------------------------------------
def build_program(repeat=1):
    import concourse.bacc as bacc
    import concourse.mybir as mybir
    from concourse.tile import TileContext

    f32 = mybir.dt.float32
    f16 = mybir.dt.float16
    b16 = mybir.dt.bfloat16
    op = mybir.AluOpType
    AF = mybir.ActivationFunctionType

    nc = bacc.Bacc("TRN2", target_bir_lowering=False, debug=False,
                   enable_asserts=False, num_devices=N_CORES)

    x_d = nc.dram_tensor("x", [IMGS, 3, H, W], f16, kind="ExternalInput").ap()
    out_d = nc.dram_tensor("out", [IMGS, 3, H, W], b16, kind="ExternalOutput").ap()
    w1_d = nc.dram_tensor("w1", [3, 128, 256], f16, kind="ExternalInput").ap()
    w2y_d = nc.dram_tensor("w2y", [IMGS, 128, 128], f16, kind="ExternalInput").ap()
    w2c_d = nc.dram_tensor("w2c", [IMGS, 128, 64], f16, kind="ExternalInput").ap()
    w3y_d = nc.dram_tensor("w3y", [IMGS, 128, 128], b16, kind="ExternalInput").ap()
    w3c_d = nc.dram_tensor("w3c", [IMGS, 128, 128], b16, kind="ExternalInput").ap()
    w4y_d = nc.dram_tensor("w4y", [128, 128], b16, kind="ExternalInput").ap()
    w4cs_d = nc.dram_tensor("w4cs", [4, 128, 128], b16, kind="ExternalInput").ap()
    rho_y_d = nc.dram_tensor("rho_y", [128, 8], f32, kind="ExternalInput").ap()
    t_y_d = nc.dram_tensor("t_y", [128, 8], f32, kind="ExternalInput").ap()
    rho_c_d = nc.dram_tensor("rho_c", [128, 8], f32, kind="ExternalInput").ap()
    t_c_d = nc.dram_tensor("t_c", [128, 8], f32, kind="ExternalInput").ap()
    zyb_d = nc.dram_tensor("zyb", [128, 1], f32, kind="ExternalInput").ap()

    with TileContext(nc, trace_sim=False) as tc:
        with tc.tile_pool(name="consts", bufs=1) as cp, \
             tc.tile_pool(name="pix", bufs=6) as pixp, \
             tc.tile_pool(name="h1", bufs=8) as h1p, \
             tc.tile_pool(name="qq", bufs=9) as qp, \
             tc.tile_pool(name="tmp", bufs=4) as tp, \
             tc.tile_pool(name="r2", bufs=6) as r2p, \
             tc.tile_pool(name="zz", bufs=5) as zp, \
             tc.tile_pool(name="outp", bufs=6) as op_, \
             tc.tile_pool(name="ps", bufs=1, space="PSUM") as pp:

            def cload(ap_dram, shape, tag, dt_=b16):
                t = cp.tile(shape, dt_, tag=tag, name=tag)
                nc.sync.dma_start(out=t[:], in_=ap_dram)
                return t

            w1_s = [cload(w1_d[p], [128, 256], f"w1{p}", f16) for p in range(3)]
            w2y_s = [cload(w2y_d[m], [128, 128], f"w2y{m}", f16)
                     for m in range(IMGS)]
            w2c_s = [cload(w2c_d[m], [128, 64], f"w2c{m}", f16)
                     for m in range(IMGS)]
            w3y_s = [cload(w3y_d[m], [128, 128], f"w3y{m}") for m in range(IMGS)]
            w3c_s = [cload(w3c_d[m], [128, 128], f"w3c{m}") for m in range(IMGS)]
            w4y_s = cload(w4y_d, [128, 128], "w4y")
            w4cs_s = [cload(w4cs_d[k], [128, 128], f"w4cs{k}") for k in range(4)]
            rho_y_s = cload(rho_y_d, [128, 8], "rho_y", f32)
            t_y_s = cload(t_y_d, [128, 8], "t_y", f32)
            rho_c_s = cload(rho_c_d, [128, 8], "rho_c", f32)
            t_c_s = cload(t_c_d, [128, 8], "t_c", f32)
            zyb_s = cload(zyb_d, [128, 1], "zyb", f32)

            def bcast8(t):  # [128,8] const -> [128,64,8] step-0 broadcast (==512)
                return t[:, None, :].broadcast_to((128, 64, 8))

            def bcast8w(t):  # wide variant (==1024)
                return t[:, None, :].broadcast_to((128, 128, 8))

            def mm(out, lhsT, rhs, **kw):
                nc.tensor.matmul(out, lhsT=lhsT, rhs=rhs, **kw)

            def quant_front(ps_tile, rho_s, qw, half):
                """Evict psum coeffs as q = c*(1/T) into half of a wide tile."""
                nc.vector.tensor_tensor(out=qw[:, 512 * half:512 * (half + 1)],
                                        in0=ps_tile[:], in1=bcast8(rho_s),
                                        op=op.mult)

            def quant_back(qs):
                """Batched diff-round on wide [128,1024] tiles: stages run
                across all tiles so each engine always has independent work.
                qs: list of (q_wide, t_s); returns wide bf16 r2 tiles."""
                n = len(qs)
                t1s = [tp.tile([128, 1024], f32, tag="t1", name="t1") for _ in range(n)]
                dps = [tp.tile([128, 1024], f16, tag="dp", name="dp") for _ in range(n)]
                d2s = [tp.tile([128, 1024], f16, tag="d2", name="d2") for _ in range(n)]
                gs = [tp.tile([128, 1024], f16, tag="g", name="g") for _ in range(n)]
                rs = [tp.tile([128, 1024], f32, tag="r", name="r") for _ in range(n)]
                r2s = [r2p.tile([128, 1024], b16, tag="r2", name="r2")
                       for _ in range(n)]
                # t1 = q + MAGIC (fp32 store rounds -> MAGIC + round(q))
                for k, (q, _) in enumerate(qs):
                    nc.scalar.activation(out=t1s[k][:], in_=q[:], func=AF.Copy,
                                         bias=MAGIC)
                # dp = (t1 - MAGIC) - q = round(q) - q = -d
                for k, (q, _) in enumerate(qs):
                    nc.vector.scalar_tensor_tensor(
                        out=dps[k][:], in0=t1s[k][:], scalar=-MAGIC, in1=q[:],
                        op0=op.add, op1=op.subtract)
                for k in range(n):
                    nc.scalar.square(out=d2s[k][:], in_=dps[k][:])
                # g = (d2-1)*dp = d - d^3
                for k in range(n):
                    nc.vector.scalar_tensor_tensor(
                        out=gs[k][:], in0=d2s[k][:], scalar=1.0, in1=dps[k][:],
                        op0=op.subtract, op1=op.mult)
                # r = q - g = round(q) + d^3
                for k, (q, _) in enumerate(qs):
                    nc.vector.tensor_tensor(out=rs[k][:], in0=q[:], in1=gs[k][:],
                                            op=op.subtract)
                # r2 = r * T
                for k, (_, t_s) in enumerate(qs):
                    nc.vector.tensor_tensor(out=r2s[k][:], in0=rs[k][:],
                                            in1=bcast8w(t_s), op=op.mult)
                return r2s

            def _front(m):
                """pix DMA + S1 + S2 + q-eviction for image m."""
                # ---- load pixel planes (one DMA per plane) ----
                pixpl = [pixp.tile([128, 2048], f16, tag="pix", name="pix")
                         for _ in range(3)]
                for p in range(3):
                    nc.sync.dma_start(
                        out=pixpl[p][:].rearrange("p (i c) -> p i c", i=4, c=512),
                        in_=x_d[m, p].rearrange("(i p) c -> p i c", i=4, p=128))
                pix = [[pixpl[p][:, 512 * i:512 * (i + 1)] for i in range(4)]
                       for p in range(3)]

                # ---- S1 (ds): vertical DCT (+color fold); 2 psum banks per
                # c-chunk, bank b: [i=2b: Y128 cb64 cr64 | i=2b+1: ...]
                h1y = []   # sbuf [c-chunk, Iu 512]
                h1c = []   # sbuf [c-chunk, cb-I'u 256 | cr-I'u 256]
                for j in range(4):
                    banks = [pp.tile([128, 512], f32, tag="psA", name="psS1", bufs=2)
                             for _ in range(2)]
                    for i in range(4):
                        bank = banks[i // 2]
                        o0 = 256 * (i % 2)
                        for p in range(3):
                            mm(bank[:, o0:o0 + 256],
                               lhsT=pix[p][i][:, 128 * j:128 * (j + 1)],
                               rhs=w1_s[p][:],
                               start=(p == 0), stop=(p == 2))
                    ty = h1p.tile([128, 512], f16, tag="h1y", name="h1y")
                    tch = h1p.tile([128, 512], f16, tag="h1c", name="h1c")
                    for b in range(2):
                        v = banks[b][:].rearrange("p (i s) -> p i s", i=2, s=256)
                        nc.scalar.copy(
                            out=ty[:].rearrange("p (i s) -> p i s", i=4, s=128)
                                [:, 2 * b:2 * b + 2, :],
                            in_=v[:, :, 0:128])
                        nc.scalar.copy(
                            out=tch[:].rearrange("p (c i v) -> p c i v",
                                                 c=2, i=4, v=64)
                                [:, :, 2 * b:2 * b + 2, :],
                            in_=v[:, :, 128:256].rearrange("p i (c v) -> p c i v",
                                                           c=2, v=64))
                    h1y.append(ty)
                    h1c.append(tch)

                # ---- S2 (std): horizontal DCT -> coeffs [(J,v), (I,u)] ----
                qw = [qp.tile([128, 1024], f32, tag="q", name="q")
                      for _ in range(3)]
                for j in range(4):
                    psQ = pp.tile([128, 512], f32, tag="psB", name="psQ", bufs=2)
                    mm(psQ[:], lhsT=w2y_s[m][:], rhs=h1y[j][:],
                                     start=True, stop=True)
                    quant_front(psQ, rho_y_s, qw[j // 2], j % 2)
                # chroma: one [128,512] psum per j-pair b; rows 0:64 = cb,
                # 64:128 = cr (partition-offset matmul writes)
                for b in range(2):
                    psQ = pp.tile([128, 512], f32, tag="psB", name="psQc", bufs=2)
                    for ch in range(2):
                        for jj in range(2):
                            j = 2 * b + jj
                            mm(psQ[64 * ch:64 * ch + 64,
                                   256 * jj:256 * (jj + 1)],
                               lhsT=w2c_s[m][:],
                               rhs=h1c[j][:, 256 * ch:256 * (ch + 1)],
                               start=True, stop=True)
                    quant_front(psQ, rho_c_s, qw[2], b)
                return [(qw[0], t_y_s), (qw[1], t_y_s), (qw[2], t_c_s)]

            def _qb(m, qt):
                """diff-round for image m -> (r2y, r2cc) wide-tile views."""
                r2w = quant_back(qt)
                r2y = [r2w[j // 2][:, 512 * (j % 2):512 * (j % 2 + 1)]
                       for j in range(4)]
                r2cc = [r2w[2][:, 512 * b:512 * (b + 1)] for b in range(2)]
                return r2y, r2cc

            def _s34(m, r2y, r2cc):
                """S3 + S4 + store for image m."""

                # ---- S3 (ds): horizontal iDCT (+h-upsample) -> [(I,u), c] ----
                zy = []
                for i in range(4):
                    psZ = pp.tile([128, 512], f32, tag="psC", name="psZ", bufs=2)
                    for j in range(4):
                        mm(psZ[:, 128 * j:128 * (j + 1)],
                                         lhsT=r2y[j][:, 128 * i:128 * (i + 1)],
                                         rhs=w3y_s[m][:], start=True, stop=True)
                    t_ = zp.tile([128, 512], b16, tag="zy", name="zy")
                    # eviction carries the decode-side +128-on-Y as a
                    # per-partition bias on u==0 rows
                    nc.scalar.activation(out=t_[:], in_=psZ[:], func=AF.Identity,
                                         bias=zyb_s[:])
                    zy.append(t_)
                # chroma Z [I'u, c]: per channel 2 tiles (I'u-chunks)
                zc = [[], []]
                for ch in range(2):
                    for k in range(2):
                        psZ = pp.tile([128, 512], f32, tag="psC", name="psZc",
                                      bufs=2)
                        po = 64 * ch
                        for j in range(4):
                            fo = 256 * (j % 2) + 128 * k
                            mm(
                                psZ[:, 128 * j:128 * (j + 1)],
                                lhsT=r2cc[j // 2][po:po + 64, fo:fo + 128],
                                rhs=w3c_s[m][po:po + 64, :],
                                start=True, stop=True)
                        t_ = zp.tile([128, 512], b16, tag="zc", name="zc")
                        nc.scalar.copy(out=t_[:], in_=psZ[:])
                        zc[ch].append(t_)

                # ---- S4 (std): vertical iDCT with YCC->RGB folded into the
                # PSUM accumulation (chroma weights pre-scaled by the mix
                # coefficients), then clamp + store ----
                outpl = [op_.tile([128, 2048], b16, tag="o", name="o")
                         for _ in range(3)]
                for i in range(4):
                    po = 64 * (i % 2)
                    zcb = zc[0][i // 2][po:po + 64, :]
                    zcr = zc[1][i // 2][po:po + 64, :]
                    def clamp(pl, ps):
                        if pl == 2:
                            # ACT evicts to bf16 (frees the PSUM bank fast);
                            # the clamp then runs in DVE 2x packed mode
                            ev = op_.tile([128, 512], b16, tag="ev", name="ev")
                            nc.scalar.copy(out=ev[:], in_=ps[:])
                            nc.vector.tensor_scalar(
                                out=outpl[pl][:, 512 * i:512 * (i + 1)],
                                in0=ev[:], scalar1=0.0, scalar2=1.0,
                                op0=op.max, op1=op.min)
                        else:
                            nc.vector.tensor_scalar(
                                out=outpl[pl][:, 512 * i:512 * (i + 1)],
                                in0=ps[:], scalar1=0.0, scalar2=1.0,
                                op0=op.max, op1=op.min)
                    psR = pp.tile([128, 512], f32, tag="psD", name="psR", bufs=2)
                    psG = pp.tile([128, 512], f32, tag="psD", name="psG", bufs=2)
                    mm(psR[:], lhsT=w4y_s[:], rhs=zy[i][:], start=True, stop=False)
                    mm(psR[:], lhsT=w4cs_s[0][po:po + 64, :], rhs=zcr,
                       start=False, stop=True)
                    mm(psG[:], lhsT=w4y_s[:], rhs=zy[i][:], start=True, stop=False)
                    mm(psG[:], lhsT=w4cs_s[1][po:po + 64, :], rhs=zcb,
                       start=False, stop=False)
                    clamp(0, psR)
                    mm(psG[:], lhsT=w4cs_s[2][po:po + 64, :], rhs=zcr,
                       start=False, stop=True)
                    psB = pp.tile([128, 512], f32, tag="psD", name="psB", bufs=2)
                    mm(psB[:], lhsT=w4y_s[:], rhs=zy[i][:], start=True, stop=False)
                    mm(psB[:], lhsT=w4cs_s[3][po:po + 64, :], rhs=zcb,
                       start=False, stop=True)
                    clamp(1, psG)
                    clamp(2, psB)
                for pl in range(3):
                    nc.sync.dma_start(
                        out=out_d[m, pl].rearrange("(i p) c -> p i c", i=4, p=128),
                        in_=outpl[pl][:].rearrange("p (i c) -> p i c", i=4, c=512))

            def _build_images():
                # three-deep software pipeline: diff-round of image m is
                # emitted ahead of image m+2's front so its DVE/ACT work
                # overlaps two images' worth of PE time before S3(m) needs it
                qts = [None] * IMGS
                r2s = [None] * IMGS
                qts[0] = _front(0)
                qts[1] = _front(1)
                r2s[0] = _qb(0, qts[0])
                for m in range(IMGS):
                    if m + 2 < IMGS:
                        qts[m + 2] = _front(m + 2)
                    _s34(m, *r2s[m])
                    r2s[m] = qts[m] = None
                    if m + 1 < IMGS:
                        r2s[m + 1] = _qb(m + 1, qts[m + 1])

            if repeat == 1:
                _build_images()
            else:
                with tc.For_i(0, repeat, 1):
                    _build_images()
    nc.compile()
    return nc


_NC_CACHE = {}


def _get_nc():
    if "nc" not in _NC_CACHE:
        _NC_CACHE["nc"] = build_program()
    return _NC_CACHE["nc"]


def kernel(x, quality):
    """Full inputs -> full output. Shards batch over 8 cores internally."""
    from concourse import bass_utils
    x = np.asarray(x, dtype=np.float32)
    quality = np.asarray(quality, dtype=np.float32)
    B = x.shape[0]
    assert B == N_CORES * IMGS, (B, N_CORES, IMGS)
    nc = _get_nc()
    in_maps = []
    for c in range(N_CORES):
        sl = slice(c * IMGS, (c + 1) * IMGS)
        in_maps.append(build_core_inputs(x[sl], quality[sl]))
    res = bass_utils.run_bass_kernel_spmd(nc, in_maps, core_ids=list(range(N_CORES)))
    outs = [np.asarray(res.results[c]["out"]).astype(np.float32)
            for c in range(N_CORES)]
    return np.concatenate(outs, axis=0)
